# revision 3
# baseline (speedup 1.0000x reference)
"""Trainium2 Bass kernel for nn_AttentiveTransformer (topk_masking).

Math: the reference's nonstandard "sparsemax" is degenerate. With ascending
sort s and f(j) = 1 + j*s_j - cumsum(s)_j, f is non-decreasing in j
(f(j)-f(j-1) = (j-1)(s_j - s_{j-1}) >= 0) and f(D-1) >= 1 > 0 always, so
k_z = D-1 = 255 for every row. Hence

    sparsemax(z) = relu(z - (rowsum(z) + 1) / 255)

and the whole module reduces to

    x  = a @ W.T                 (+b cancels inside ghost BN)
    xn = ghost_bn(x) * bn_w + bn_b        (per 128-row chunk)
    z  = xn * prior_scales
    m  = relu(z - (rowsum(z)+1)/255)
    new_prior = prior_scales * (1.5 - m)

Layout: pure data parallel over 8 cores (16384 rows each). Batch rows on
SBUF partitions, features on the free dim. One BN chunk == one 128-row tile,
so BN stats are per-feature partition-dim sums, done on the TensorEngine
(one-hot selector matmuls into a [16,256] PSUM stats tile per supertile of
16 chunks). Chunk means of `a` are folded in on the host (centering `a`
before the matmul makes x - mean exact), so only E[xc^2] is computed on
device. The per-chunk rsqrt row is broadcast across partitions with a K=16
one-hot matmul, and the normalize + prior multiply + row-sum run as a single
fused scalar_tensor_tensor with accum_out.
"""

import numpy as np

_NC = 8
_N, _NA, _F, _VBS = 131072, 128, 256, 128
_GAMMA, _EPS = 1.5, 1e-5
_G = 16                       # chunks per supertile
_R = _N // _NC                # rows per core = 16384
_CH = _R // _VBS              # chunks per core = 128
_ST = _CH // _G               # supertiles per core = 8

_prog_cache = {}
LAST_RESULTS = None           # BassKernelResults of the most recent run


def _build(has_prior, has_bnb, trace=False):
    from contextlib import ExitStack
    import concourse.bass as bass
    import concourse.bacc as bacc
    import concourse.tile as tile
    from concourse import mybir
    from concourse.alu_op_type import AluOpType as op

    f32 = mybir.dt.float32
    AF = mybir.ActivationFunctionType

    nc = bacc.Bacc("TRN2", debug=False, target_bir_lowering=False,
                   num_devices=_NC)

    aT_d = nc.declare_dram_parameter("aT", [_NA, _R], f32, isOutput=False)
    abar_d = nc.declare_dram_parameter("abar", [_NA, _CH], f32, isOutput=False)
    Wt_d = nc.declare_dram_parameter("Wt", [_NA, _F], f32, isOutput=False)
    bnw_d = nc.declare_dram_parameter("bnw", [_G, _F], f32, isOutput=False)
    Z_d = nc.declare_dram_parameter("Zsel", [_VBS, 2 * _G], f32, isOutput=False)
    OH_d = nc.declare_dram_parameter("OH", [_G, _G * _VBS], f32, isOutput=False)
    if has_prior:
        prior_d = nc.declare_dram_parameter("prior", [_R, _F], f32, isOutput=False)
    if has_bnb:
        bnb_d = nc.declare_dram_parameter("bnb", [_VBS, _F], f32, isOutput=False)
    m_d = nc.declare_dram_parameter("m_out", [_R, _F], f32, isOutput=True)
    np_d = nc.declare_dram_parameter("np_out", [_R, _F], f32, isOutput=True)

    with tile.TileContext(nc) as tc, ExitStack() as ctx:
        singles = ctx.enter_context(tc.tile_pool(name="singles", bufs=1))
        at_pool = ctx.enter_context(tc.tile_pool(name="at", bufs=2))
        xcs_pool = ctx.enter_context(tc.tile_pool(name="xcs", bufs=2))
        atc_pool = ctx.enter_context(tc.tile_pool(name="atc", bufs=3))
        sq_pool = ctx.enter_context(tc.tile_pool(name="sq", bufs=3))
        z_pool = ctx.enter_context(tc.tile_pool(name="z", bufs=4))
        m_pool = ctx.enter_context(tc.tile_pool(name="m", bufs=4))
        npr_pool = ctx.enter_context(tc.tile_pool(name="npr", bufs=4))
        small_pool = ctx.enter_context(tc.tile_pool(name="small", bufs=8))
        stat_pool = ctx.enter_context(tc.tile_pool(name="stat", bufs=2))
        if has_prior:
            pr_pool = ctx.enter_context(tc.tile_pool(name="pr", bufs=3))
            gp_pool = ctx.enter_context(tc.tile_pool(name="gp", bufs=3))
        psum_x = ctx.enter_context(tc.tile_pool(name="psx", bufs=3, space="PSUM"))
        psum_g = ctx.enter_context(tc.tile_pool(name="psg", bufs=2, space="PSUM"))
        psum_s = ctx.enter_context(tc.tile_pool(name="pss", bufs=2, space="PSUM"))

        Wt_sb = singles.tile([_NA, _F], f32)
        nc.sync.dma_start(Wt_sb[:], Wt_d[:])
        abar_sb = singles.tile([_NA, _CH], f32)
        nc.sync.dma_start(abar_sb[:], abar_d[:])
        bnw_sb = singles.tile([_G, _F], f32)
        nc.sync.dma_start(bnw_sb[:], bnw_d[:])
        Z_sb = singles.tile([_VBS, 2 * _G], f32)
        nc.sync.dma_start(Z_sb[:], Z_d[:])
        OH_sb = singles.tile([_G, _G * _VBS], f32)
        nc.sync.dma_start(OH_sb[:], OH_d[:])
        if has_bnb:
            bnb_sb = singles.tile([_VBS, _F], f32)
            nc.sync.dma_start(bnb_sb[:], bnb_d[:])
        eps_sb = singles.tile([_G, 1], f32)
        nc.vector.memset(eps_sb[:], float(_EPS))

        for s in range(_ST):
            at_sb = at_pool.tile([_NA, _G * _VBS], f32)
            nc.sync.dma_start(at_sb[:], aT_d[:, s * _G * _VBS:(s + 1) * _G * _VBS])
            xcs = xcs_pool.tile([_VBS, _G * _F], f32)
            statq = psum_s.tile([_G, _F], f32)

            # phase 1: centered matmul + per-chunk sum(xc^2) into statq rows
            for c in range(_G):
                gc = s * _G + c
                atc = atc_pool.tile([_NA, _VBS], f32)
                nc.vector.tensor_scalar_sub(
                    atc[:], at_sb[:, c * _VBS:(c + 1) * _VBS], abar_sb[:, gc:gc + 1])
                xp = psum_x.tile([_VBS, _F], f32)
                nc.tensor.matmul(xp[:], atc[:], Wt_sb[:], start=True, stop=True)
                sq = sq_pool.tile([_VBS, _F], f32)
                nc.scalar.activation(sq[:], xp[:], AF.Square)
                nc.tensor.matmul(statq[:], Z_sb[:, _G - c:2 * _G - c], sq[:],
                                 start=(c == 0), stop=(c == _G - 1))
                nc.vector.tensor_copy(xcs[:, c * _F:(c + 1) * _F], xp[:])

            # stats: rsq = bn_w / sqrt(statq/128 + eps)   [G, F]
            sd = stat_pool.tile([_G, _F], f32)
            nc.scalar.activation(sd[:], statq[:], AF.Sqrt,
                                 bias=eps_sb[:], scale=1.0 / _VBS)
            rsq = stat_pool.tile([_G, _F], f32)
            nc.vector.reciprocal(rsq[:], sd[:])
            rsqw = stat_pool.tile([_G, _F], f32)
            nc.vector.tensor_tensor(rsqw[:], rsq[:], bnw_sb[:], op.mult)

            # phase 2: broadcast rsq row, normalize, sparsemax, outputs
            for c in range(_G):
                gc = s * _G + c
                gb = psum_g.tile([_VBS, _F], f32)
                nc.tensor.matmul(gb[:], OH_sb[:, c * _VBS:(c + 1) * _VBS],
                                 rsqw[:], start=True, stop=True)
                z = z_pool.tile([_VBS, _F], f32)
                rs = small_pool.tile([_VBS, 1], f32)
                xc_sl = xcs[:, c * _F:(c + 1) * _F]
                if has_prior:
                    pr = pr_pool.tile([_VBS, _F], f32)
                    nc.sync.dma_start(
                        pr[:], prior_d[gc * _VBS:(gc + 1) * _VBS, :])
                    if has_bnb:
                        xn = gp_pool.tile([_VBS, _F], f32)
                        nc.vector.scalar_tensor_tensor(
                            xn[:], xc_sl, 0.0, gb[:], op.add, op.mult)
                        xnb = gp_pool.tile([_VBS, _F], f32)
                        nc.vector.tensor_tensor(xnb[:], xn[:], bnb_sb[:], op.add)
                        nc.vector.scalar_tensor_tensor(
                            z[:], xnb[:], 0.0, pr[:], op.add, op.mult,
                            accum_out=rs[:])
                    else:
                        gp = gp_pool.tile([_VBS, _F], f32)
                        nc.vector.tensor_tensor(gp[:], pr[:], gb[:], op.mult)
                        nc.vector.scalar_tensor_tensor(
                            z[:], xc_sl, 0.0, gp[:], op.add, op.mult,
                            accum_out=rs[:])
                else:
                    if has_bnb:
                        xn = z_pool.tile([_VBS, _F], f32)
                        nc.vector.scalar_tensor_tensor(
                            xn[:], xc_sl, 0.0, gb[:], op.add, op.mult)
                        nc.vector.scalar_tensor_tensor(
                            z[:], xn[:], 0.0, bnb_sb[:], op.add, op.add,
                            accum_out=rs[:])
                    else:
                        nc.vector.scalar_tensor_tensor(
                            z[:], xc_sl, 0.0, gb[:], op.add, op.mult,
                            accum_out=rs[:])
                taun = small_pool.tile([_VBS, 1], f32)
                nc.vector.tensor_scalar(taun[:], rs[:], 1.0, -1.0 / 255.0,
                                        op.add, op.mult)
                mt = m_pool.tile([_VBS, _F], f32)
                nc.scalar.activation(mt[:], z[:], AF.Relu, bias=taun[:], scale=1.0)
                nt = npr_pool.tile([_VBS, _F], f32)
                if has_prior:
                    gm = npr_pool.tile([_VBS, _F], f32)
                    nc.gpsimd.tensor_scalar(gm[:], mt[:], -1.0, _GAMMA,
                                            op.mult, op.add)
                    nc.gpsimd.tensor_tensor(nt[:], gm[:], pr[:], op.mult)
                else:
                    nc.gpsimd.tensor_scalar(nt[:], mt[:], -1.0, _GAMMA,
                                            op.mult, op.add)
                nc.sync.dma_start(m_d[gc * _VBS:(gc + 1) * _VBS, :], mt[:])
                nc.sync.dma_start(np_d[gc * _VBS:(gc + 1) * _VBS, :], nt[:])

    nc.compile()
    return nc


def kernel(a, prior_scales, W, b, bn_weight, bn_bias, _trace=False):
    global LAST_RESULTS
    from concourse.bass_utils import run_bass_kernel_spmd

    a = np.ascontiguousarray(np.asarray(a, dtype=np.float32))
    prior_scales = np.ascontiguousarray(np.asarray(prior_scales, dtype=np.float32))
    W = np.asarray(W, dtype=np.float32)
    bn_weight = np.asarray(bn_weight, dtype=np.float32)
    bn_bias = np.asarray(bn_bias, dtype=np.float32)
    # b cancels exactly inside ghost BN (shifts x and the chunk mean equally,
    # and leaves the variance unchanged), so it is not sent to the device.

    has_prior = not bool(np.all(prior_scales == np.float32(1.0)))
    has_bnb = bool(np.any(bn_bias != 0.0))

    key = (has_prior, has_bnb)
    if key not in _prog_cache:
        _prog_cache[key] = _build(has_prior, has_bnb)
    nc = _prog_cache[key]

    # host-side prep (layout only, plus cheap chunk means of `a`)
    aT = np.ascontiguousarray(a.T)                                # [128, N]
    abar = np.ascontiguousarray(
        a.reshape(_N // _VBS, _VBS, _NA).mean(axis=1, dtype=np.float64)
        .astype(np.float32).T)                                    # [128, 1024]
    Wt = np.ascontiguousarray(W.T)                                # [128, 256]
    bnw16 = np.ascontiguousarray(
        np.broadcast_to(bn_weight[None, :], (_G, _F)).astype(np.float32))
    Zsel = np.zeros((_VBS, 2 * _G), np.float32)
    Zsel[:, _G] = 1.0
    OH = np.kron(np.eye(_G, dtype=np.float32),
                 np.ones((1, _VBS), np.float32))                  # [16, 2048]

    in_maps = []
    for i in range(_NC):
        d = {
            "aT": np.ascontiguousarray(aT[:, i * _R:(i + 1) * _R]),
            "abar": np.ascontiguousarray(abar[:, i * _CH:(i + 1) * _CH]),
            "Wt": Wt,
            "bnw": bnw16,
            "Zsel": Zsel,
            "OH": OH,
        }
        if has_prior:
            d["prior"] = np.ascontiguousarray(prior_scales[i * _R:(i + 1) * _R])
        if has_bnb:
            d["bnb"] = np.ascontiguousarray(
                np.broadcast_to(bn_bias[None, :], (_VBS, _F)).astype(np.float32))
        in_maps.append(d)

    LAST_RESULTS = run_bass_kernel_spmd(nc, in_maps, list(range(_NC)),
                                        trace=_trace)
    res = LAST_RESULTS.results
    m = np.concatenate([res[i]["m_out"] for i in range(_NC)], axis=0)
    new_prior = np.concatenate([res[i]["np_out"] for i in range(_NC)], axis=0)
    return m, new_prior


# revision 7
# speedup vs baseline: 1.3188x; 1.3188x over previous
"""Trainium2 Bass kernel for nn_AttentiveTransformer (topk_masking).

Math: the reference's nonstandard "sparsemax" is degenerate. With ascending
sort s and f(j) = 1 + j*s_j - cumsum(s)_j, f is non-decreasing in j
(f(j)-f(j-1) = (j-1)(s_j - s_{j-1}) >= 0) and f(D-1) >= 1 > 0 always, so
k_z = D-1 = 255 for every row. Hence

    sparsemax(z) = relu(z - (rowsum(z) + 1) / 255)

and the whole module reduces to

    x  = a @ W.T                 (+b cancels inside ghost BN)
    xn = ghost_bn(x) * bn_w + bn_b        (per 128-row chunk)
    z  = xn * prior_scales
    m  = relu(z - (rowsum(z)+1)/255)
    new_prior = prior_scales * (1.5 - m)

Layout: pure data parallel over 8 cores (16384 rows each). Batch rows on
SBUF partitions, features on the free dim. One BN chunk == one 128-row tile,
so BN stats are per-feature partition-dim sums, done on the TensorEngine
(one-hot selector matmuls into a [16,256] PSUM stats tile per supertile of
16 chunks). Chunk means of `a` are folded in on the host (centering `a`
before the matmul makes x - mean exact), so only E[xc^2] is computed on
device. The per-chunk rsqrt row is broadcast across partitions with a K=16
one-hot matmul, and the normalize + prior multiply + row-sum run as a single
fused scalar_tensor_tensor with accum_out.
"""

import numpy as np

_NC = 8
_N, _NA, _F, _VBS = 131072, 128, 256, 128
_GAMMA, _EPS = 1.5, 1e-5
_G = 16                       # chunks per supertile
_R = _N // _NC                # rows per core = 16384
_CH = _R // _VBS              # chunks per core = 128
_ST = _CH // _G               # supertiles per core = 8

_prog_cache = {}
LAST_RESULTS = None           # BassKernelResults of the most recent run


def _build(has_prior, has_bnb, use_f32r=True):
    from contextlib import ExitStack
    import concourse.bass as bass
    import concourse.bacc as bacc
    import concourse.tile as tile
    from concourse import mybir
    from concourse.alu_op_type import AluOpType as op

    f32 = mybir.dt.float32
    # f32r: PE's rounded-fp32 mode (TF32-like, ~1.5e-4 rel err) at 1 cy/row
    # for N>=256 vs 4 cy/row for fp32, and with background weight loads.
    fmm = mybir.dt.float32r if use_f32r else f32
    AF = mybir.ActivationFunctionType

    nc = bacc.Bacc("TRN2", debug=False, target_bir_lowering=False,
                   num_devices=_NC)

    aT_d = nc.declare_dram_parameter("aT", [_NA, _R], f32, isOutput=False)
    abar_d = nc.declare_dram_parameter("abar", [_NA, _CH], f32, isOutput=False)
    Wt_d = nc.declare_dram_parameter("Wt", [_NA, _F], f32, isOutput=False)
    bnw_d = nc.declare_dram_parameter("bnw", [_G, _F], f32, isOutput=False)
    Z_d = nc.declare_dram_parameter("Zsel", [_VBS, 2 * _G], f32, isOutput=False)
    OH_d = nc.declare_dram_parameter("OH", [_G, _G * _VBS], f32, isOutput=False)
    if has_prior:
        prior_d = nc.declare_dram_parameter("prior", [_R, _F], f32, isOutput=False)
    if has_bnb:
        bnb_d = nc.declare_dram_parameter("bnb", [_VBS, _F], f32, isOutput=False)
    m_d = nc.declare_dram_parameter("m_out", [_R, _F], f32, isOutput=True)
    np_d = nc.declare_dram_parameter("np_out", [_R, _F], f32, isOutput=True)

    with tile.TileContext(nc) as tc, ExitStack() as ctx:
        singles = ctx.enter_context(tc.tile_pool(name="singles", bufs=1))
        at_pool = ctx.enter_context(tc.tile_pool(name="at", bufs=2))
        xcs_pool = ctx.enter_context(tc.tile_pool(name="xcs", bufs=2))
        atc_pool = ctx.enter_context(tc.tile_pool(name="atc", bufs=3))
        sq_pool = ctx.enter_context(tc.tile_pool(name="sq", bufs=3))
        z_pool = ctx.enter_context(tc.tile_pool(name="z", bufs=4))
        m_pool = ctx.enter_context(tc.tile_pool(name="m", bufs=4))
        npr_pool = ctx.enter_context(tc.tile_pool(name="npr", bufs=4))
        small_pool = ctx.enter_context(tc.tile_pool(name="small", bufs=8))
        stat_pool = ctx.enter_context(tc.tile_pool(name="stat", bufs=2))
        if has_prior:
            pr_pool = ctx.enter_context(tc.tile_pool(name="pr", bufs=3))
            gp_pool = ctx.enter_context(tc.tile_pool(name="gp", bufs=3))
        psum_x = ctx.enter_context(tc.tile_pool(name="psx", bufs=3, space="PSUM"))
        psum_g = ctx.enter_context(tc.tile_pool(name="psg", bufs=2, space="PSUM"))
        psum_s = ctx.enter_context(tc.tile_pool(name="pss", bufs=2, space="PSUM"))

        Wt_f32 = singles.tile([_NA, _F], f32)
        nc.sync.dma_start(Wt_f32[:], Wt_d[:])
        abar_sb = singles.tile([_NA, _CH], f32)
        nc.sync.dma_start(abar_sb[:], abar_d[:])
        bnw_sb = singles.tile([_G, _F], f32)
        nc.sync.dma_start(bnw_sb[:], bnw_d[:])
        Z_f32 = singles.tile([_VBS, 2 * _G], f32)
        nc.sync.dma_start(Z_f32[:], Z_d[:])
        OH_f32 = singles.tile([_G, _G * _VBS], f32)
        nc.sync.dma_start(OH_f32[:], OH_d[:])
        if use_f32r:
            # one-time casts so the BIR verifier sees f32r-rounded producers
            Wt_sb = singles.tile([_NA, _F], fmm)
            nc.vector.tensor_copy(Wt_sb[:], Wt_f32[:])
            Z_sb = singles.tile([_VBS, 2 * _G], fmm)
            nc.vector.tensor_copy(Z_sb[:], Z_f32[:])
            OH_sb = singles.tile([_G, _G * _VBS], fmm)
            nc.vector.tensor_copy(OH_sb[:], OH_f32[:])
        else:
            Wt_sb, Z_sb, OH_sb = Wt_f32, Z_f32, OH_f32
        if has_bnb:
            bnb_sb = singles.tile([_VBS, _F], f32)
            nc.sync.dma_start(bnb_sb[:], bnb_d[:])
        eps_sb = singles.tile([_G, 1], f32)
        nc.vector.memset(eps_sb[:], float(_EPS))

        for s in range(_ST):
            at_sb = at_pool.tile([_NA, _G * _VBS], f32)
            nc.sync.dma_start(at_sb[:], aT_d[:, s * _G * _VBS:(s + 1) * _G * _VBS])
            xcs = xcs_pool.tile([_VBS, _G * _F], f32)
            statq = psum_s.tile([_G, _F], f32)

            # phase 1: centered matmul + per-chunk sum(xc^2) into statq rows
            for c in range(_G):
                gc = s * _G + c
                atc = atc_pool.tile([_NA, _VBS], fmm)
                nc.vector.tensor_scalar_sub(
                    atc[:], at_sb[:, c * _VBS:(c + 1) * _VBS], abar_sb[:, gc:gc + 1])
                xp = psum_x.tile([_VBS, _F], f32)
                nc.tensor.matmul(xp[:], atc[:], Wt_sb[:], start=True, stop=True)
                sq = sq_pool.tile([_VBS, _F], fmm)
                nc.scalar.activation(sq[:], xp[:], AF.Square)
                nc.tensor.matmul(statq[:], Z_sb[:, _G - c:2 * _G - c], sq[:],
                                 start=(c == 0), stop=(c == _G - 1))
                nc.vector.tensor_copy(xcs[:, c * _F:(c + 1) * _F], xp[:])

            # stats: rsq = bn_w / sqrt(statq/128 + eps)   [G, F]
            sd = stat_pool.tile([_G, _F], f32)
            nc.scalar.activation(sd[:], statq[:], AF.Sqrt,
                                 bias=eps_sb[:], scale=1.0 / _VBS)
            rsq = stat_pool.tile([_G, _F], f32)
            nc.vector.reciprocal(rsq[:], sd[:])
            rsqw = stat_pool.tile([_G, _F], fmm)
            nc.vector.tensor_tensor(rsqw[:], rsq[:], bnw_sb[:], op.mult)

            # phase 2: broadcast rsq row, normalize, sparsemax, outputs
            for c in range(_G):
                gc = s * _G + c
                gb = psum_g.tile([_VBS, _F], f32)
                nc.tensor.matmul(gb[:], OH_sb[:, c * _VBS:(c + 1) * _VBS],
                                 rsqw[:], start=True, stop=True)
                z = z_pool.tile([_VBS, _F], f32)
                rs = small_pool.tile([_VBS, 1], f32)
                xc_sl = xcs[:, c * _F:(c + 1) * _F]
                if has_prior:
                    pr = pr_pool.tile([_VBS, _F], f32)
                    nc.sync.dma_start(
                        pr[:], prior_d[gc * _VBS:(gc + 1) * _VBS, :])
                    if has_bnb:
                        xn = gp_pool.tile([_VBS, _F], f32)
                        nc.vector.scalar_tensor_tensor(
                            xn[:], xc_sl, 0.0, gb[:], op.add, op.mult)
                        xnb = gp_pool.tile([_VBS, _F], f32)
                        nc.vector.tensor_tensor(xnb[:], xn[:], bnb_sb[:], op.add)
                        nc.vector.scalar_tensor_tensor(
                            z[:], xnb[:], 0.0, pr[:], op.add, op.mult,
                            accum_out=rs[:])
                    else:
                        gp = gp_pool.tile([_VBS, _F], f32)
                        nc.vector.tensor_tensor(gp[:], pr[:], gb[:], op.mult)
                        nc.vector.scalar_tensor_tensor(
                            z[:], xc_sl, 0.0, gp[:], op.add, op.mult,
                            accum_out=rs[:])
                else:
                    if has_bnb:
                        xn = z_pool.tile([_VBS, _F], f32)
                        nc.vector.scalar_tensor_tensor(
                            xn[:], xc_sl, 0.0, gb[:], op.add, op.mult)
                        nc.vector.scalar_tensor_tensor(
                            z[:], xn[:], 0.0, bnb_sb[:], op.add, op.add,
                            accum_out=rs[:])
                    else:
                        nc.vector.scalar_tensor_tensor(
                            z[:], xc_sl, 0.0, gb[:], op.add, op.mult,
                            accum_out=rs[:])
                taun = small_pool.tile([_VBS, 1], f32)
                nc.vector.tensor_scalar(taun[:], rs[:], 1.0, -1.0 / 255.0,
                                        op.add, op.mult)
                mt = m_pool.tile([_VBS, _F], f32)
                nc.scalar.activation(mt[:], z[:], AF.Relu, bias=taun[:], scale=1.0)
                nt = npr_pool.tile([_VBS, _F], f32)
                if has_prior:
                    gm = npr_pool.tile([_VBS, _F], f32)
                    nc.gpsimd.tensor_scalar(gm[:], mt[:], -1.0, _GAMMA,
                                            op.mult, op.add)
                    nc.gpsimd.tensor_tensor(nt[:], gm[:], pr[:], op.mult)
                else:
                    nc.gpsimd.tensor_scalar(nt[:], mt[:], -1.0, _GAMMA,
                                            op.mult, op.add)
                nc.sync.dma_start(m_d[gc * _VBS:(gc + 1) * _VBS, :], mt[:])
                nc.sync.dma_start(np_d[gc * _VBS:(gc + 1) * _VBS, :], nt[:])

    nc.compile()
    return nc


def kernel(a, prior_scales, W, b, bn_weight, bn_bias, _trace=False):
    global LAST_RESULTS
    from concourse.bass_utils import run_bass_kernel_spmd

    a = np.ascontiguousarray(np.asarray(a, dtype=np.float32))
    prior_scales = np.ascontiguousarray(np.asarray(prior_scales, dtype=np.float32))
    W = np.asarray(W, dtype=np.float32)
    bn_weight = np.asarray(bn_weight, dtype=np.float32)
    bn_bias = np.asarray(bn_bias, dtype=np.float32)
    # b cancels exactly inside ghost BN (shifts x and the chunk mean equally,
    # and leaves the variance unchanged), so it is not sent to the device.

    has_prior = not bool(np.all(prior_scales == np.float32(1.0)))
    has_bnb = bool(np.any(bn_bias != 0.0))

    key = (has_prior, has_bnb)
    if key not in _prog_cache:
        _prog_cache[key] = _build(has_prior, has_bnb)
    nc = _prog_cache[key]

    # host-side prep (layout only, plus cheap chunk means of `a`)
    aT = np.ascontiguousarray(a.T)                                # [128, N]
    abar = np.ascontiguousarray(
        a.reshape(_N // _VBS, _VBS, _NA).mean(axis=1, dtype=np.float64)
        .astype(np.float32).T)                                    # [128, 1024]
    Wt = np.ascontiguousarray(W.T)                                # [128, 256]
    bnw16 = np.ascontiguousarray(
        np.broadcast_to(bn_weight[None, :], (_G, _F)).astype(np.float32))
    Zsel = np.zeros((_VBS, 2 * _G), np.float32)
    Zsel[:, _G] = 1.0
    OH = np.kron(np.eye(_G, dtype=np.float32),
                 np.ones((1, _VBS), np.float32))                  # [16, 2048]

    in_maps = []
    for i in range(_NC):
        d = {
            "aT": np.ascontiguousarray(aT[:, i * _R:(i + 1) * _R]),
            "abar": np.ascontiguousarray(abar[:, i * _CH:(i + 1) * _CH]),
            "Wt": Wt,
            "bnw": bnw16,
            "Zsel": Zsel,
            "OH": OH,
        }
        if has_prior:
            d["prior"] = np.ascontiguousarray(prior_scales[i * _R:(i + 1) * _R])
        if has_bnb:
            d["bnb"] = np.ascontiguousarray(
                np.broadcast_to(bn_bias[None, :], (_VBS, _F)).astype(np.float32))
        in_maps.append(d)

    LAST_RESULTS = run_bass_kernel_spmd(nc, in_maps, list(range(_NC)),
                                        trace=_trace)
    res = LAST_RESULTS.results
    m = np.concatenate([res[i]["m_out"] for i in range(_NC)], axis=0)
    new_prior = np.concatenate([res[i]["np_out"] for i in range(_NC)], axis=0)
    return m, new_prior
